# revision 12
# baseline (speedup 1.0000x reference)
"""Entmax-alpha (bisection+log-secant) Trainium2 kernel.

Full inputs: att_scores [4,16,1024,1024] f32, alpha [16] f32.
Reference: p = entmax_bisect(att_scores, a) with a = 1.01 + 0.98*alpha per
head, 50 bisection iterations over the last axis (K=1024), then p/sum(p).

Algorithm (per row): find tau with sum(relu(am1*x - tau)^inv) = 1 via
1 bisection probe + clamped secant iterations on f = ln(s).  The secant on
ln(s) is far better conditioned than on s (ln s is exactly linear in tau in
the alpha->1 softmax limit), so 5 total evaluations reach ~2e-4 global rel
error vs the 50-iteration reference, and 4 reach ~4e-3 (gate is 2e-2).

Key implementation points:
- x-space: tau' = tau/am1.  y = relu(x - tau') on DVE; the ACT Ln applies
  scale=am1 (per-partition) so ln(am1*y + 1e-30) needs no xs=x*am1 tensor.
- ACT does 2 passes/eval: one 4-tile-wide Ln (4096+4 elems amortize the
  ~352-cycle instruction overhead) + 4 per-tile Exp with accum_out giving
  the row sums s for free.  ln(s) for the secant is a tiny [128,4] Ln.
- All per-row scalar state (tau_lo, tau_m, hi, s, f) for the 4 chains of a
  group lives in [128,4] tiles: one DVE op updates all 4 chains.
- Groups are emitted pairwise-interleaved so group A's DVE update+relu runs
  under group B's ACT Ln/Exp: ACT stays ~100% busy (it is the bottleneck:
  ~10k cycles/group/eval vs ~7k DVE).
- Secant guard: den = (f_prev - f) + 1e-30 (order matters: the 1e-30 must
  be added AFTER the subtract so an exact-zero den is impossible), and
  reciprocal+clamp keep every intermediate finite - NaN cannot occur.

Sharding: 64 (b,h) slices, embarrassingly parallel -> 8 contiguous slices
per core across 8 NeuronCores.
"""

import numpy as np

import concourse.bacc as bacc
import concourse.tile as tile
from concourse import mybir
import concourse.bass as bass
from concourse.bass_utils import run_bass_kernel_spmd

F32 = np.float32

B, H, Q, K = 4, 16, 1024, 1024
N_CORES = 8
SLICES_PER_CORE = (B * H) // N_CORES  # 8
ROW_TILES = Q // 128  # 8
GRP = 4  # row-tile chains per group (one wide Ln covers the group)

N_BIS = 1  # bisection probes
N_SEC = 3  # clamped log-secant evaluations (last one is the output eval)
N_EVALS = N_BIS + N_SEC
# scal columns: 0=am1, 1=inv, 2=neg_inv, 3=c0(=dm_1-inv), 4..=dm_1..dm_N
M_COLS = 4 + N_EVALS

_AF = mybir.ActivationFunctionType
_OP = mybir.AluOpType

# --- ACT table-set selection fix -------------------------------------------
# Bacc's insert_act_table_loads picks, per activation, the FIRST table set
# containing that function: Ln -> "natural_log", Exp -> "exp_and_others".
# Alternating Ln/Exp then reloads tables every iteration (~2.7us each).
# Both live in "natural_log_exp_and_others"; strip Ln/Exp from every other
# set so the pass hoists a single load of the combined set.
_COMBINED_SET = "natural_log_exp_and_others"
_orig_gat = bacc.get_activation_tables


def _patched_gat(arch):
    tabs = _orig_gat(arch)
    out = {}
    for n, funcs in tabs.items():
        f = set(funcs)
        if n != _COMBINED_SET:
            f.discard(_AF.Ln)
            f.discard(_AF.Exp)
        out[n] = f
    return out


bacc.get_activation_tables = _patched_gat
# ---------------------------------------------------------------------------


def _build_nc(n_bis: int = N_BIS, n_sec: int = N_SEC):
    n_evals = n_bis + n_sec
    nc = bacc.Bacc("TRN2", target_bir_lowering=False, debug=False)
    x = nc.dram_tensor(
        "x", [SLICES_PER_CORE, Q, K], mybir.dt.float32, kind="ExternalInput"
    )
    scal = nc.dram_tensor(
        "scal", [SLICES_PER_CORE, M_COLS], mybir.dt.float32, kind="ExternalInput"
    )
    out = nc.dram_tensor(
        "out", [SLICES_PER_CORE, Q, K], mybir.dt.float32, kind="ExternalOutput"
    )

    with tile.TileContext(nc) as tc:
        with (
            tc.tile_pool(name="consts", bufs=1) as consts,
            tc.tile_pool(name="xr", bufs=6 * GRP) as xr_pool,
            tc.tile_pool(name="y", bufs=4) as y_pool,
            tc.tile_pool(name="p", bufs=2 * GRP) as p_pool,  # final p -> out
            tc.tile_pool(name="small", bufs=8) as small,  # per-tag rings of 8
        ):
            scal_sb = consts.tile([128, SLICES_PER_CORE, M_COLS], mybir.dt.float32)
            scal_bcast = bass.AP(
                tensor=scal[:].tensor,
                offset=0,
                ap=[[0, 128], [M_COLS, SLICES_PER_CORE], [1, M_COLS]],
            )
            nc.sync.dma_start(out=scal_sb, in_=scal_bcast)

            tiny = consts.tile([128, 1], mybir.dt.float32)
            nc.vector.memset(tiny, 1e-30)

            def col(s, c):  # scal column c of slice s as [128,1] AP
                return scal_sb[:, s, c : c + 1]

            def dm_ap(s, i):  # dm_i of slice s, i in 1..n_evals
                return col(s, 3 + i)

            groups = [
                (s, g0) for s in range(SLICES_PER_CORE) for g0 in range(0, ROW_TILES, GRP)
            ]

            # chain state per live group, keyed by group index
            st = {}

            DMA_CHUNKS = 4  # col-chunks per tile -> more queues in parallel

            def emit_dma(gi):
                """Queue the input DMAs for group gi, split into column chunks
                so one tile's data spreads across several DMA queues (a single
                queue sustains only ~1/16 of HBM bandwidth)."""
                s, t0 = groups[gi]
                c = {"x": []}
                ck = K // DMA_CHUNKS
                for j in range(GRP):
                    rows = slice((t0 + j) * 128, (t0 + j + 1) * 128)
                    x_t = xr_pool.tile([128, K], mybir.dt.float32, tag="xr")
                    for q in range(DMA_CHUNKS):
                        cols = slice(q * ck, (q + 1) * ck)
                        nc.sync.dma_start(out=x_t[:, cols], in_=x[s, rows, cols])
                    c["x"].append(x_t)
                st[gi] = c

            def emit_prep(gi):
                """DVE part of setup: row maxes + initial tau.  Emitted well
                after emit_dma(gi): DVE is in-order, so these must not enter
                the queue until their DMAs are about to complete."""
                s, t0 = groups[gi]
                c = st[gi]
                mx4 = small.tile([128, GRP], mybir.dt.float32, tag="mx0")
                for j in range(GRP):
                    nc.vector.reduce_max(
                        out=mx4[:, j : j + 1], in_=c["x"][j], axis=mybir.AxisListType.X
                    )
                tau_lo = small.tile([128, GRP], mybir.dt.float32, tag="tlo0")
                nc.vector.tensor_scalar_add(out=tau_lo, in0=mx4, scalar1=col(s, 2))
                tau_m = small.tile([128, GRP], mybir.dt.float32, tag="tm0")
                nc.vector.tensor_scalar_add(out=tau_m, in0=mx4, scalar1=col(s, 3))
                c["tlo"], c["tm"] = tau_lo, tau_m
                c["tm_prev"] = c["f_prev"] = c["hi"] = None

            def emit_relu(gi, i):
                """DVE part of eval i: y = relu(x - tau).  Emitted separately
                so the next pair's first relus can jump ahead of the previous
                pair's normalize tail in the in-order DVE queue."""
                c = st[gi]
                y_g = y_pool.tile([128, GRP * K], mybir.dt.float32, tag="y")
                for j in range(GRP):
                    nc.vector.tensor_scalar(
                        out=y_g[:, j * K : (j + 1) * K],
                        in0=c["x"][j],
                        scalar1=c["tm"][:, j : j + 1],
                        scalar2=0.0,
                        op0=_OP.subtract,
                        op1=_OP.max,
                    )
                c["y"] = y_g

            def emit_rest(gi, i):
                """ACT Ln + 4x Exp(+accum) and, for non-final evals, the
                bracket/secant update."""
                s, t0 = groups[gi]
                c = st[gi]
                final = i == n_evals
                y_g = c.pop("y")
                l_g = y_g  # in-place: y is dead once Ln has streamed it
                nc.scalar.activation(
                    out=l_g, in_=y_g, func=_AF.Ln, bias=tiny, scale=col(s, 0)
                )
                s4 = small.tile([128, GRP], mybir.dt.float32, tag="s")
                if final:
                    p_ts = [
                        p_pool.tile([128, K], mybir.dt.float32, tag="p", name=f"pf{j}")
                        for j in range(GRP)
                    ]
                else:
                    p_ts = [l_g[:, j * K : (j + 1) * K] for j in range(GRP)]
                for j in range(GRP):
                    nc.scalar.activation(
                        out=p_ts[j],
                        in_=l_g[:, j * K : (j + 1) * K],
                        func=_AF.Exp,
                        bias=0.0,
                        scale=col(s, 1),
                        accum_out=s4[:, j : j + 1],
                    )
                if final:
                    c["p_ts"], c["s4"] = p_ts, s4
                    return

                f4 = None
                if i >= n_bis:  # ln(s) feeds this or the next secant update
                    f4 = small.tile([128, GRP], mybir.dt.float32, tag="f")
                    nc.scalar.activation(
                        out=f4, in_=s4, func=_AF.Ln, bias=tiny, scale=1.0
                    )
                cond = small.tile([128, GRP], mybir.dt.float32, tag="c")
                nc.vector.tensor_scalar(
                    out=cond, in0=s4, scalar1=1.0, scalar2=None, op0=_OP.is_ge
                )
                if i <= n_bis:
                    # bisect: lo += cond*dm_i ; (i==n_bis: hi = lo + dm_i)
                    tau_lo2 = small.tile([128, GRP], mybir.dt.float32, tag="tlo")
                    nc.vector.scalar_tensor_tensor(
                        out=tau_lo2,
                        in0=cond,
                        scalar=dm_ap(s, i),
                        in1=c["tlo"],
                        op0=_OP.mult,
                        op1=_OP.add,
                    )
                    if i == n_bis:
                        hi = small.tile([128, GRP], mybir.dt.float32, tag="hi")
                        nc.vector.tensor_scalar_add(
                            out=hi, in0=tau_lo2, scalar1=dm_ap(s, i)
                        )
                        c["hi"] = hi
                    tau_m2 = small.tile([128, GRP], mybir.dt.float32, tag="tm")
                    nc.vector.tensor_scalar_add(
                        out=tau_m2, in0=tau_lo2, scalar1=dm_ap(s, i + 1)
                    )
                    c["tlo"] = tau_lo2
                else:
                    # clamped log-secant through (tm_prev, f_prev), (tm, f)
                    cbar = small.tile([128, GRP], mybir.dt.float32, tag="cb")
                    nc.vector.tensor_scalar(
                        out=cbar, in0=s4, scalar1=1.0, scalar2=None, op0=_OP.is_lt
                    )
                    nc.vector.copy_predicated(
                        out=c["tlo"], mask=cond.bitcast(mybir.dt.uint32), data=c["tm"]
                    )
                    nc.vector.copy_predicated(
                        out=c["hi"], mask=cbar.bitcast(mybir.dt.uint32), data=c["tm"]
                    )
                    dtau = small.tile([128, GRP], mybir.dt.float32, tag="w0")
                    nc.vector.tensor_sub(out=dtau, in0=c["tm"], in1=c["tm_prev"])
                    den = small.tile([128, GRP], mybir.dt.float32, tag="w1")
                    nc.vector.tensor_sub(out=den, in0=c["f_prev"], in1=f4)
                    nc.vector.tensor_scalar_add(out=den, in0=den, scalar1=1e-30)
                    rden = small.tile([128, GRP], mybir.dt.float32, tag="w2")
                    nc.vector.reciprocal(out=rden, in_=den)
                    q = small.tile([128, GRP], mybir.dt.float32, tag="w3")
                    nc.vector.tensor_mul(out=q, in0=f4, in1=rden)
                    qd = small.tile([128, GRP], mybir.dt.float32, tag="w4")
                    nc.vector.tensor_mul(out=qd, in0=q, in1=dtau)
                    ts_ = small.tile([128, GRP], mybir.dt.float32, tag="w5")
                    nc.vector.tensor_add(out=ts_, in0=qd, in1=c["tm"])
                    tc_ = small.tile([128, GRP], mybir.dt.float32, tag="w6")
                    nc.vector.tensor_max(out=tc_, in0=ts_, in1=c["tlo"])
                    tau_m2 = small.tile([128, GRP], mybir.dt.float32, tag="tm")
                    nc.vector.tensor_tensor(
                        out=tau_m2, in0=tc_, in1=c["hi"], op=_OP.min
                    )
                c["tm_prev"], c["f_prev"] = c["tm"], f4
                c["tm"] = tau_m2

            def emit_tail(gi):
                """Normalize + store the final eval's p (after the next pair's
                first relus, so those hit the DVE queue first)."""
                s, t0 = groups[gi]
                c = st[gi]
                p_ts, s4 = c["p_ts"], c["s4"]
                r4 = small.tile([128, GRP], mybir.dt.float32, tag="r")
                nc.vector.reciprocal(out=r4, in_=s4)
                ck = K // DMA_CHUNKS
                for j in range(GRP):
                    rows = slice((t0 + j) * 128, (t0 + j + 1) * 128)
                    nc.vector.tensor_scalar_mul(
                        out=p_ts[j], in0=p_ts[j], scalar1=r4[:, j : j + 1]
                    )
                    for q in range(DMA_CHUNKS):
                        cols = slice(q * ck, (q + 1) * ck)
                        nc.sync.dma_start(out=out[s, rows, cols], in_=p_ts[j][:, cols])
                del st[gi]

            # Pairwise interleave: groups (2k, 2k+1) advance eval-by-eval so
            # one group's DVE phase hides under the other's ACT phase.
            # Staggered startup: group 0's DMA+prep+relu go first so the first
            # Ln is not gated on the whole initial DMA burst.
            emit_dma(0)
            emit_prep(0)
            emit_relu(0, 1)
            emit_dma(1)
            emit_prep(1)
            emit_relu(1, 1)
            emit_dma(2)
            emit_dma(3)
            n_g = len(groups)
            for k in range(0, n_g, 2):
                for g in (k + 4, k + 5):  # two pairs ahead of their prep
                    if g < n_g:
                        emit_dma(g)
                for i in range(1, n_evals + 1):
                    if i > 1:
                        emit_relu(k, i)
                    emit_rest(k, i)
                    if i > 1:
                        emit_relu(k + 1, i)
                    emit_rest(k + 1, i)
                    if i == 2:
                        for g in (k + 2, k + 3):
                            if g < n_g:
                                emit_prep(g)
                # next pair's first relus outrank the normalize tails on DVE
                if k + 2 < n_g:
                    emit_relu(k + 2, 1)
                    emit_relu(k + 3, 1)
                emit_tail(k)
                emit_tail(k + 1)

    nc.finalize()
    return nc


_NC_CACHE = {}


def _get_nc():
    key = (N_BIS, N_SEC)
    if key not in _NC_CACHE:
        _NC_CACHE[key] = _build_nc(*key)
    return _NC_CACHE[key]


def _host_scal_table(alpha: np.ndarray) -> np.ndarray:
    """Per-(b,h)-slice constant table (x-space), f32 math."""
    a = (F32(1.01) + F32(0.98) * alpha.astype(F32)).astype(F32)  # [H]
    am1 = (a - F32(1.0)).astype(F32)
    inv = (F32(1.0) / am1).astype(F32)
    dm0p = ((F32(1.0) - (F32(1.0 / K) ** inv)) * inv).astype(F32)  # x-space width
    tab = np.zeros((B * H, M_COLS), dtype=F32)
    for g in range(B * H):
        h = g % H
        dms = [F32(dm0p[h] * F32(0.5) ** i) for i in range(1, N_EVALS + 2)]
        tab[g, 0] = am1[h]
        tab[g, 1] = inv[h]
        tab[g, 2] = -inv[h]
        tab[g, 3] = F32(dms[0] - inv[h])  # c0 = dm_1 - inv
        for i in range(1, N_EVALS + 1):
            tab[g, 3 + i] = dms[i - 1]
    return tab


def kernel(att_scores: np.ndarray, alpha: np.ndarray, **run_kwargs) -> np.ndarray:
    assert att_scores.shape == (B, H, Q, K), att_scores.shape
    nc = _get_nc()
    xr = np.ascontiguousarray(att_scores, dtype=np.float32).reshape(B * H, Q, K)
    tab = _host_scal_table(np.asarray(alpha))
    in_maps = []
    for c in range(N_CORES):
        sl = slice(c * SLICES_PER_CORE, (c + 1) * SLICES_PER_CORE)
        in_maps.append(
            {
                "x": np.ascontiguousarray(xr[sl]),
                "scal": np.ascontiguousarray(tab[sl]),
            }
        )
    res = run_bass_kernel_spmd(nc, in_maps, core_ids=list(range(N_CORES)), **run_kwargs)
    outs = np.stack([res.results[c]["out"] for c in range(N_CORES)], axis=0)
    full = outs.reshape(B, H, Q, K).astype(np.float32)
    if run_kwargs:
        kernel.last_result = res
    return full


# revision 13
# speedup vs baseline: 1.0377x; 1.0377x over previous
"""Entmax-alpha (bisection+log-secant) Trainium2 kernel.

Full inputs: att_scores [4,16,1024,1024] f32, alpha [16] f32.
Reference: p = entmax_bisect(att_scores, a) with a = 1.01 + 0.98*alpha per
head, 50 bisection iterations over the last axis (K=1024), then p/sum(p).

Algorithm (per row): find tau with sum(relu(am1*x - tau)^inv) = 1 via
1 bisection probe + clamped secant iterations on f = ln(s).  The secant on
ln(s) is far better conditioned than on s (ln s is exactly linear in tau in
the alpha->1 softmax limit), so 5 total evaluations reach ~2e-4 global rel
error vs the 50-iteration reference, and 4 reach ~4e-3 (gate is 2e-2).

Key implementation points:
- x-space: tau' = tau/am1.  y = relu(x - tau') on DVE; the ACT Ln applies
  scale=am1 (per-partition) so ln(am1*y + 1e-30) needs no xs=x*am1 tensor.
- ACT does 2 passes/eval: one 4-tile-wide Ln (4096+4 elems amortize the
  ~352-cycle instruction overhead) + 4 per-tile Exp with accum_out giving
  the row sums s for free.  ln(s) for the secant is a tiny [128,4] Ln.
- All per-row scalar state (tau_lo, tau_m, hi, s, f) for the 4 chains of a
  group lives in [128,4] tiles: one DVE op updates all 4 chains.
- Groups are emitted pairwise-interleaved so group A's DVE update+relu runs
  under group B's ACT Ln/Exp: ACT stays ~100% busy (it is the bottleneck:
  ~10k cycles/group/eval vs ~7k DVE).
- Secant guard: den = (f_prev - f) + 1e-30 (order matters: the 1e-30 must
  be added AFTER the subtract so an exact-zero den is impossible), and
  reciprocal+clamp keep every intermediate finite - NaN cannot occur.

Sharding: 64 (b,h) slices, embarrassingly parallel -> 8 contiguous slices
per core across 8 NeuronCores.
"""

import numpy as np

import concourse.bacc as bacc
import concourse.tile as tile
from concourse import mybir
import concourse.bass as bass
from concourse.bass_utils import run_bass_kernel_spmd

F32 = np.float32

B, H, Q, K = 4, 16, 1024, 1024
N_CORES = 8
SLICES_PER_CORE = (B * H) // N_CORES  # 8
ROW_TILES = Q // 128  # 8
GRP = 4  # row-tile chains per group (one wide Ln covers the group)

N_BIS = 1  # bisection probes
N_SEC = 3  # clamped log-secant evaluations (last one is the output eval)
N_EVALS = N_BIS + N_SEC
# scal columns: 0=am1, 1=inv, 2=neg_inv, 3=c0(=dm_1-inv), 4..=dm_1..dm_N
M_COLS = 4 + N_EVALS

_AF = mybir.ActivationFunctionType
_OP = mybir.AluOpType

# --- ACT table-set selection fix -------------------------------------------
# Bacc's insert_act_table_loads picks, per activation, the FIRST table set
# containing that function: Ln -> "natural_log", Exp -> "exp_and_others".
# Alternating Ln/Exp then reloads tables every iteration (~2.7us each).
# Both live in "natural_log_exp_and_others"; strip Ln/Exp from every other
# set so the pass hoists a single load of the combined set.
_COMBINED_SET = "natural_log_exp_and_others"
_orig_gat = bacc.get_activation_tables


def _patched_gat(arch):
    tabs = _orig_gat(arch)
    out = {}
    for n, funcs in tabs.items():
        f = set(funcs)
        if n != _COMBINED_SET:
            f.discard(_AF.Ln)
            f.discard(_AF.Exp)
        out[n] = f
    return out


bacc.get_activation_tables = _patched_gat
# ---------------------------------------------------------------------------


def _build_nc(n_bis: int = N_BIS, n_sec: int = N_SEC):
    n_evals = n_bis + n_sec
    nc = bacc.Bacc("TRN2", target_bir_lowering=False, debug=False)
    x = nc.dram_tensor(
        "x", [SLICES_PER_CORE, Q, K], mybir.dt.float32, kind="ExternalInput"
    )
    scal = nc.dram_tensor(
        "scal", [SLICES_PER_CORE, M_COLS], mybir.dt.float32, kind="ExternalInput"
    )
    out = nc.dram_tensor(
        "out", [SLICES_PER_CORE, Q, K], mybir.dt.float32, kind="ExternalOutput"
    )

    with tile.TileContext(nc) as tc:
        with (
            tc.tile_pool(name="consts", bufs=1) as consts,
            tc.tile_pool(name="xr", bufs=6 * GRP) as xr_pool,
            tc.tile_pool(name="y", bufs=4) as y_pool,
            tc.tile_pool(name="p", bufs=2 * GRP) as p_pool,  # final p -> out
            tc.tile_pool(name="small", bufs=8) as small,  # per-tag rings of 8
        ):
            scal_sb = consts.tile([128, SLICES_PER_CORE, M_COLS], mybir.dt.float32)
            scal_bcast = bass.AP(
                tensor=scal[:].tensor,
                offset=0,
                ap=[[0, 128], [M_COLS, SLICES_PER_CORE], [1, M_COLS]],
            )
            nc.sync.dma_start(out=scal_sb, in_=scal_bcast)

            tiny = consts.tile([128, 1], mybir.dt.float32)
            nc.vector.memset(tiny, 1e-30)

            def col(s, c):  # scal column c of slice s as [128,1] AP
                return scal_sb[:, s, c : c + 1]

            def dm_ap(s, i):  # dm_i of slice s, i in 1..n_evals
                return col(s, 3 + i)

            groups = [
                (s, g0) for s in range(SLICES_PER_CORE) for g0 in range(0, ROW_TILES, GRP)
            ]

            # chain state per live group, keyed by group index
            st = {}

            def emit_dma(gi, chunks=1):
                """Queue the input DMAs for group gi.  chunks>1 splits each
                tile across several DMA queues: lower arrival latency, lower
                per-queue efficiency - only worth it for the startup groups."""
                s, t0 = groups[gi]
                c = {"x": []}
                ck = K // chunks
                for j in range(GRP):
                    rows = slice((t0 + j) * 128, (t0 + j + 1) * 128)
                    x_t = xr_pool.tile([128, K], mybir.dt.float32, tag="xr")
                    for q in range(chunks):
                        cols = slice(q * ck, (q + 1) * ck)
                        nc.sync.dma_start(out=x_t[:, cols], in_=x[s, rows, cols])
                    c["x"].append(x_t)
                st[gi] = c

            def emit_prep(gi):
                """DVE part of setup: row maxes + initial tau.  Emitted well
                after emit_dma(gi): DVE is in-order, so these must not enter
                the queue until their DMAs are about to complete."""
                s, t0 = groups[gi]
                c = st[gi]
                mx4 = small.tile([128, GRP], mybir.dt.float32, tag="mx0")
                for j in range(GRP):
                    nc.vector.reduce_max(
                        out=mx4[:, j : j + 1], in_=c["x"][j], axis=mybir.AxisListType.X
                    )
                tau_lo = small.tile([128, GRP], mybir.dt.float32, tag="tlo0")
                nc.vector.tensor_scalar_add(out=tau_lo, in0=mx4, scalar1=col(s, 2))
                tau_m = small.tile([128, GRP], mybir.dt.float32, tag="tm0")
                nc.vector.tensor_scalar_add(out=tau_m, in0=mx4, scalar1=col(s, 3))
                c["tlo"], c["tm"] = tau_lo, tau_m
                c["tm_prev"] = c["f_prev"] = c["hi"] = None

            def emit_relu(gi, i):
                """DVE part of eval i: y = relu(x - tau).  Emitted separately
                so the next pair's first relus can jump ahead of the previous
                pair's normalize tail in the in-order DVE queue."""
                c = st[gi]
                y_g = y_pool.tile([128, GRP * K], mybir.dt.float32, tag="y")
                for j in range(GRP):
                    nc.vector.tensor_scalar(
                        out=y_g[:, j * K : (j + 1) * K],
                        in0=c["x"][j],
                        scalar1=c["tm"][:, j : j + 1],
                        scalar2=0.0,
                        op0=_OP.subtract,
                        op1=_OP.max,
                    )
                c["y"] = y_g

            def emit_rest(gi, i):
                """ACT Ln + 4x Exp(+accum) and, for non-final evals, the
                bracket/secant update."""
                s, t0 = groups[gi]
                c = st[gi]
                final = i == n_evals
                y_g = c.pop("y")
                l_g = y_g  # in-place: y is dead once Ln has streamed it
                nc.scalar.activation(
                    out=l_g, in_=y_g, func=_AF.Ln, bias=tiny, scale=col(s, 0)
                )
                s4 = small.tile([128, GRP], mybir.dt.float32, tag="s")
                if final:
                    p_ts = [
                        p_pool.tile([128, K], mybir.dt.float32, tag="p", name=f"pf{j}")
                        for j in range(GRP)
                    ]
                else:
                    p_ts = [l_g[:, j * K : (j + 1) * K] for j in range(GRP)]
                for j in range(GRP):
                    nc.scalar.activation(
                        out=p_ts[j],
                        in_=l_g[:, j * K : (j + 1) * K],
                        func=_AF.Exp,
                        bias=0.0,
                        scale=col(s, 1),
                        accum_out=s4[:, j : j + 1],
                    )
                if final:
                    c["p_ts"], c["s4"] = p_ts, s4
                    return

                f4 = None
                if i >= n_bis:  # ln(s) feeds this or the next secant update
                    f4 = small.tile([128, GRP], mybir.dt.float32, tag="f")
                    nc.scalar.activation(
                        out=f4, in_=s4, func=_AF.Ln, bias=tiny, scale=1.0
                    )
                cond = small.tile([128, GRP], mybir.dt.float32, tag="c")
                nc.vector.tensor_scalar(
                    out=cond, in0=s4, scalar1=1.0, scalar2=None, op0=_OP.is_ge
                )
                if i <= n_bis:
                    # bisect: lo += cond*dm_i ; (i==n_bis: hi = lo + dm_i)
                    tau_lo2 = small.tile([128, GRP], mybir.dt.float32, tag="tlo")
                    nc.vector.scalar_tensor_tensor(
                        out=tau_lo2,
                        in0=cond,
                        scalar=dm_ap(s, i),
                        in1=c["tlo"],
                        op0=_OP.mult,
                        op1=_OP.add,
                    )
                    if i == n_bis:
                        hi = small.tile([128, GRP], mybir.dt.float32, tag="hi")
                        nc.vector.tensor_scalar_add(
                            out=hi, in0=tau_lo2, scalar1=dm_ap(s, i)
                        )
                        c["hi"] = hi
                    tau_m2 = small.tile([128, GRP], mybir.dt.float32, tag="tm")
                    nc.vector.tensor_scalar_add(
                        out=tau_m2, in0=tau_lo2, scalar1=dm_ap(s, i + 1)
                    )
                    c["tlo"] = tau_lo2
                else:
                    # clamped log-secant through (tm_prev, f_prev), (tm, f)
                    cbar = small.tile([128, GRP], mybir.dt.float32, tag="cb")
                    nc.vector.tensor_scalar(
                        out=cbar, in0=s4, scalar1=1.0, scalar2=None, op0=_OP.is_lt
                    )
                    nc.vector.copy_predicated(
                        out=c["tlo"], mask=cond.bitcast(mybir.dt.uint32), data=c["tm"]
                    )
                    nc.vector.copy_predicated(
                        out=c["hi"], mask=cbar.bitcast(mybir.dt.uint32), data=c["tm"]
                    )
                    dtau = small.tile([128, GRP], mybir.dt.float32, tag="w0")
                    nc.vector.tensor_sub(out=dtau, in0=c["tm"], in1=c["tm_prev"])
                    den = small.tile([128, GRP], mybir.dt.float32, tag="w1")
                    nc.vector.tensor_sub(out=den, in0=c["f_prev"], in1=f4)
                    nc.vector.tensor_scalar_add(out=den, in0=den, scalar1=1e-30)
                    rden = small.tile([128, GRP], mybir.dt.float32, tag="w2")
                    nc.vector.reciprocal(out=rden, in_=den)
                    q = small.tile([128, GRP], mybir.dt.float32, tag="w3")
                    nc.vector.tensor_mul(out=q, in0=f4, in1=rden)
                    qd = small.tile([128, GRP], mybir.dt.float32, tag="w4")
                    nc.vector.tensor_mul(out=qd, in0=q, in1=dtau)
                    ts_ = small.tile([128, GRP], mybir.dt.float32, tag="w5")
                    nc.vector.tensor_add(out=ts_, in0=qd, in1=c["tm"])
                    tc_ = small.tile([128, GRP], mybir.dt.float32, tag="w6")
                    nc.vector.tensor_max(out=tc_, in0=ts_, in1=c["tlo"])
                    tau_m2 = small.tile([128, GRP], mybir.dt.float32, tag="tm")
                    nc.vector.tensor_tensor(
                        out=tau_m2, in0=tc_, in1=c["hi"], op=_OP.min
                    )
                c["tm_prev"], c["f_prev"] = c["tm"], f4
                c["tm"] = tau_m2

            def emit_tail(gi):
                """Normalize + store the final eval's p (after the next pair's
                first relus, so those hit the DVE queue first)."""
                s, t0 = groups[gi]
                c = st[gi]
                p_ts, s4 = c["p_ts"], c["s4"]
                r4 = small.tile([128, GRP], mybir.dt.float32, tag="r")
                nc.vector.reciprocal(out=r4, in_=s4)
                for j in range(GRP):
                    rows = slice((t0 + j) * 128, (t0 + j + 1) * 128)
                    nc.vector.tensor_scalar_mul(
                        out=p_ts[j], in0=p_ts[j], scalar1=r4[:, j : j + 1]
                    )
                    nc.sync.dma_start(out=out[s, rows, :], in_=p_ts[j])
                del st[gi]

            # Pairwise interleave: groups (2k, 2k+1) advance eval-by-eval so
            # one group's DVE phase hides under the other's ACT phase.
            # Staggered startup: group 0's DMA+prep+relu go first so the first
            # Ln is not gated on the whole initial DMA burst.
            emit_dma(0, chunks=4)
            emit_prep(0)
            emit_relu(0, 1)
            emit_dma(1, chunks=4)
            emit_prep(1)
            emit_relu(1, 1)
            emit_dma(2)
            emit_dma(3)
            n_g = len(groups)
            for k in range(0, n_g, 2):
                for g in (k + 4, k + 5):  # two pairs ahead of their prep
                    if g < n_g:
                        emit_dma(g)
                for i in range(1, n_evals + 1):
                    if i > 1:
                        emit_relu(k, i)
                    emit_rest(k, i)
                    if i > 1:
                        emit_relu(k + 1, i)
                    emit_rest(k + 1, i)
                    if i == 2:
                        for g in (k + 2, k + 3):
                            if g < n_g:
                                emit_prep(g)
                # next pair's first relus outrank the normalize tails on DVE
                if k + 2 < n_g:
                    emit_relu(k + 2, 1)
                    emit_relu(k + 3, 1)
                emit_tail(k)
                emit_tail(k + 1)

    nc.finalize()
    return nc


_NC_CACHE = {}


def _get_nc():
    key = (N_BIS, N_SEC)
    if key not in _NC_CACHE:
        _NC_CACHE[key] = _build_nc(*key)
    return _NC_CACHE[key]


def _host_scal_table(alpha: np.ndarray) -> np.ndarray:
    """Per-(b,h)-slice constant table (x-space), f32 math."""
    a = (F32(1.01) + F32(0.98) * alpha.astype(F32)).astype(F32)  # [H]
    am1 = (a - F32(1.0)).astype(F32)
    inv = (F32(1.0) / am1).astype(F32)
    dm0p = ((F32(1.0) - (F32(1.0 / K) ** inv)) * inv).astype(F32)  # x-space width
    tab = np.zeros((B * H, M_COLS), dtype=F32)
    for g in range(B * H):
        h = g % H
        dms = [F32(dm0p[h] * F32(0.5) ** i) for i in range(1, N_EVALS + 2)]
        tab[g, 0] = am1[h]
        tab[g, 1] = inv[h]
        tab[g, 2] = -inv[h]
        tab[g, 3] = F32(dms[0] - inv[h])  # c0 = dm_1 - inv
        for i in range(1, N_EVALS + 1):
            tab[g, 3 + i] = dms[i - 1]
    return tab


def kernel(att_scores: np.ndarray, alpha: np.ndarray, **run_kwargs) -> np.ndarray:
    assert att_scores.shape == (B, H, Q, K), att_scores.shape
    nc = _get_nc()
    xr = np.ascontiguousarray(att_scores, dtype=np.float32).reshape(B * H, Q, K)
    tab = _host_scal_table(np.asarray(alpha))
    in_maps = []
    for c in range(N_CORES):
        sl = slice(c * SLICES_PER_CORE, (c + 1) * SLICES_PER_CORE)
        in_maps.append(
            {
                "x": np.ascontiguousarray(xr[sl]),
                "scal": np.ascontiguousarray(tab[sl]),
            }
        )
    res = run_bass_kernel_spmd(nc, in_maps, core_ids=list(range(N_CORES)), **run_kwargs)
    outs = np.stack([res.results[c]["out"] for c in range(N_CORES)], axis=0)
    full = outs.reshape(B, H, Q, K).astype(np.float32)
    if run_kwargs:
        kernel.last_result = res
    return full


# revision 14
# speedup vs baseline: 1.0459x; 1.0079x over previous
"""Entmax-alpha (bisection+log-secant) Trainium2 kernel.

Full inputs: att_scores [4,16,1024,1024] f32, alpha [16] f32.
Reference: p = entmax_bisect(att_scores, a) with a = 1.01 + 0.98*alpha per
head, 50 bisection iterations over the last axis (K=1024), then p/sum(p).

Algorithm (per row): find tau with sum(relu(am1*x - tau)^inv) = 1 via
1 bisection probe + clamped secant iterations on f = ln(s).  The secant on
ln(s) is far better conditioned than on s (ln s is exactly linear in tau in
the alpha->1 softmax limit), so 5 total evaluations reach ~2e-4 global rel
error vs the 50-iteration reference, and 4 reach ~4e-3 (gate is 2e-2).

Key implementation points:
- x-space: tau' = tau/am1.  y = relu(x - tau') on DVE; the ACT Ln applies
  scale=am1 (per-partition) so ln(am1*y + 1e-30) needs no xs=x*am1 tensor.
- ACT does 2 passes/eval: one 4-tile-wide Ln (4096+4 elems amortize the
  ~352-cycle instruction overhead) + 4 per-tile Exp with accum_out giving
  the row sums s for free.  ln(s) for the secant is a tiny [128,4] Ln.
- All per-row scalar state (tau_lo, tau_m, hi, s, f) for the 4 chains of a
  group lives in [128,4] tiles: one DVE op updates all 4 chains.
- Groups are emitted pairwise-interleaved so group A's DVE update+relu runs
  under group B's ACT Ln/Exp: ACT stays ~100% busy (it is the bottleneck:
  ~10k cycles/group/eval vs ~7k DVE).
- Secant guard: den = (f_prev - f) + 1e-30 (order matters: the 1e-30 must
  be added AFTER the subtract so an exact-zero den is impossible), and
  reciprocal+clamp keep every intermediate finite - NaN cannot occur.

Sharding: 64 (b,h) slices, embarrassingly parallel -> 8 contiguous slices
per core across 8 NeuronCores.
"""

import numpy as np

import concourse.bacc as bacc
import concourse.tile as tile
from concourse import mybir
import concourse.bass as bass
from concourse.bass_utils import run_bass_kernel_spmd

F32 = np.float32

B, H, Q, K = 4, 16, 1024, 1024
N_CORES = 8
SLICES_PER_CORE = (B * H) // N_CORES  # 8
ROW_TILES = Q // 128  # 8
GRP = 4  # row-tile chains per group (one wide Ln covers the group)

N_BIS = 1  # bisection probes
N_SEC = 3  # clamped log-secant evaluations (last one is the output eval)
N_EVALS = N_BIS + N_SEC
# scal columns: 0=am1, 1=inv, 2=neg_inv, 3=c0(=dm_1-inv), 4..=dm_1..dm_N
M_COLS = 4 + N_EVALS

_AF = mybir.ActivationFunctionType
_OP = mybir.AluOpType

# --- ACT table-set selection fix -------------------------------------------
# Bacc's insert_act_table_loads picks, per activation, the FIRST table set
# containing that function: Ln -> "natural_log", Exp -> "exp_and_others".
# Alternating Ln/Exp then reloads tables every iteration (~2.7us each).
# Both live in "natural_log_exp_and_others"; strip Ln/Exp from every other
# set so the pass hoists a single load of the combined set.
_COMBINED_SET = "natural_log_exp_and_others"
_orig_gat = bacc.get_activation_tables


def _patched_gat(arch):
    tabs = _orig_gat(arch)
    out = {}
    for n, funcs in tabs.items():
        f = set(funcs)
        if n != _COMBINED_SET:
            f.discard(_AF.Ln)
            f.discard(_AF.Exp)
        out[n] = f
    return out


bacc.get_activation_tables = _patched_gat
# ---------------------------------------------------------------------------


def _build_nc(n_bis: int = N_BIS, n_sec: int = N_SEC):
    n_evals = n_bis + n_sec
    nc = bacc.Bacc("TRN2", target_bir_lowering=False, debug=False)
    x = nc.dram_tensor(
        "x", [SLICES_PER_CORE, Q, K], mybir.dt.float32, kind="ExternalInput"
    )
    scal = nc.dram_tensor(
        "scal", [SLICES_PER_CORE, M_COLS], mybir.dt.float32, kind="ExternalInput"
    )
    out = nc.dram_tensor(
        "out", [SLICES_PER_CORE, Q, K], mybir.dt.float32, kind="ExternalOutput"
    )

    with tile.TileContext(nc) as tc:
        with (
            tc.tile_pool(name="consts", bufs=1) as consts,
            tc.tile_pool(name="xr", bufs=6 * GRP) as xr_pool,
            tc.tile_pool(name="y", bufs=4) as y_pool,
            tc.tile_pool(name="p", bufs=2 * GRP) as p_pool,  # final p -> out
            tc.tile_pool(name="small", bufs=8) as small,  # per-tag rings of 8
        ):
            scal_sb = consts.tile([128, SLICES_PER_CORE, M_COLS], mybir.dt.float32)
            scal_bcast = bass.AP(
                tensor=scal[:].tensor,
                offset=0,
                ap=[[0, 128], [M_COLS, SLICES_PER_CORE], [1, M_COLS]],
            )
            nc.sync.dma_start(out=scal_sb, in_=scal_bcast)

            tiny = consts.tile([128, 1], mybir.dt.float32)
            nc.vector.memset(tiny, 1e-30)

            def col(s, c):  # scal column c of slice s as [128,1] AP
                return scal_sb[:, s, c : c + 1]

            def dm_ap(s, i):  # dm_i of slice s, i in 1..n_evals
                return col(s, 3 + i)

            groups = [
                (s, g0) for s in range(SLICES_PER_CORE) for g0 in range(0, ROW_TILES, GRP)
            ]

            # chain state per live group, keyed by group index
            st = {}

            def emit_dma(gi, chunks=1):
                """Queue the input DMAs for group gi.  chunks>1 splits each
                tile across several DMA queues: lower arrival latency, lower
                per-queue efficiency - only worth it for the startup groups."""
                s, t0 = groups[gi]
                c = {"x": []}
                ck = K // chunks
                for j in range(GRP):
                    rows = slice((t0 + j) * 128, (t0 + j + 1) * 128)
                    x_t = xr_pool.tile([128, K], mybir.dt.float32, tag="xr")
                    for q in range(chunks):
                        cols = slice(q * ck, (q + 1) * ck)
                        nc.sync.dma_start(out=x_t[:, cols], in_=x[s, rows, cols])
                    c["x"].append(x_t)
                st[gi] = c

            def emit_prep(gi):
                """DVE part of setup: row maxes + initial tau.  Emitted well
                after emit_dma(gi): DVE is in-order, so these must not enter
                the queue until their DMAs are about to complete."""
                s, t0 = groups[gi]
                c = st[gi]
                mx4 = small.tile([128, GRP], mybir.dt.float32, tag="mx0")
                for j in range(GRP):
                    nc.vector.reduce_max(
                        out=mx4[:, j : j + 1], in_=c["x"][j], axis=mybir.AxisListType.X
                    )
                tau_lo = small.tile([128, GRP], mybir.dt.float32, tag="tlo0")
                nc.vector.tensor_scalar_add(out=tau_lo, in0=mx4, scalar1=col(s, 2))
                tau_m = small.tile([128, GRP], mybir.dt.float32, tag="tm0")
                nc.vector.tensor_scalar_add(out=tau_m, in0=mx4, scalar1=col(s, 3))
                c["tlo"], c["tm"] = tau_lo, tau_m
                c["tm_prev"] = c["f_prev"] = c["hi"] = None

            def emit_relu(gi, i):
                """DVE part of eval i: y = relu(x - tau).  Emitted separately
                so the next pair's first relus can jump ahead of the previous
                pair's normalize tail in the in-order DVE queue."""
                c = st[gi]
                y_g = y_pool.tile([128, GRP * K], mybir.dt.float32, tag="y")
                for j in range(GRP):
                    nc.vector.tensor_scalar(
                        out=y_g[:, j * K : (j + 1) * K],
                        in0=c["x"][j],
                        scalar1=c["tm"][:, j : j + 1],
                        scalar2=0.0,
                        op0=_OP.subtract,
                        op1=_OP.max,
                    )
                c["y"] = y_g

            def emit_rest(gi, i):
                """ACT Ln + 4x Exp(+accum) and, for non-final evals, the
                bracket/secant update."""
                s, t0 = groups[gi]
                c = st[gi]
                final = i == n_evals
                y_g = c.pop("y")
                l_g = y_g  # in-place: y is dead once Ln has streamed it
                nc.scalar.activation(
                    out=l_g, in_=y_g, func=_AF.Ln, bias=tiny, scale=col(s, 0)
                )
                s4 = small.tile([128, GRP], mybir.dt.float32, tag="s")
                if final:
                    p_ts = [
                        p_pool.tile([128, K], mybir.dt.float32, tag="p", name=f"pf{j}")
                        for j in range(GRP)
                    ]
                else:
                    p_ts = [l_g[:, j * K : (j + 1) * K] for j in range(GRP)]
                for j in range(GRP):
                    nc.scalar.activation(
                        out=p_ts[j],
                        in_=l_g[:, j * K : (j + 1) * K],
                        func=_AF.Exp,
                        bias=0.0,
                        scale=col(s, 1),
                        accum_out=s4[:, j : j + 1],
                    )
                if final:
                    c["p_ts"], c["s4"] = p_ts, s4
                    return

                f4 = None
                if i >= n_bis:  # ln(s) feeds this or the next secant update
                    f4 = small.tile([128, GRP], mybir.dt.float32, tag="f")
                    nc.scalar.activation(
                        out=f4, in_=s4, func=_AF.Ln, bias=tiny, scale=1.0
                    )
                cond = small.tile([128, GRP], mybir.dt.float32, tag="c")
                nc.vector.tensor_scalar(
                    out=cond, in0=s4, scalar1=1.0, scalar2=None, op0=_OP.is_ge
                )
                if i <= n_bis:
                    # bisect: lo += cond*dm_i ; (i==n_bis: hi = lo + dm_i)
                    tau_lo2 = small.tile([128, GRP], mybir.dt.float32, tag="tlo")
                    nc.vector.scalar_tensor_tensor(
                        out=tau_lo2,
                        in0=cond,
                        scalar=dm_ap(s, i),
                        in1=c["tlo"],
                        op0=_OP.mult,
                        op1=_OP.add,
                    )
                    if i == n_bis:
                        hi = small.tile([128, GRP], mybir.dt.float32, tag="hi")
                        nc.vector.tensor_scalar_add(
                            out=hi, in0=tau_lo2, scalar1=dm_ap(s, i)
                        )
                        c["hi"] = hi
                    tau_m2 = small.tile([128, GRP], mybir.dt.float32, tag="tm")
                    nc.vector.tensor_scalar_add(
                        out=tau_m2, in0=tau_lo2, scalar1=dm_ap(s, i + 1)
                    )
                    c["tlo"] = tau_lo2
                else:
                    # clamped log-secant through (tm_prev, f_prev), (tm, f)
                    cbar = small.tile([128, GRP], mybir.dt.float32, tag="cb")
                    nc.vector.tensor_scalar(
                        out=cbar, in0=s4, scalar1=1.0, scalar2=None, op0=_OP.is_lt
                    )
                    nc.vector.copy_predicated(
                        out=c["tlo"], mask=cond.bitcast(mybir.dt.uint32), data=c["tm"]
                    )
                    nc.vector.copy_predicated(
                        out=c["hi"], mask=cbar.bitcast(mybir.dt.uint32), data=c["tm"]
                    )
                    dtau = small.tile([128, GRP], mybir.dt.float32, tag="w0")
                    nc.vector.tensor_sub(out=dtau, in0=c["tm"], in1=c["tm_prev"])
                    den = small.tile([128, GRP], mybir.dt.float32, tag="w1")
                    nc.vector.tensor_sub(out=den, in0=c["f_prev"], in1=f4)
                    nc.vector.tensor_scalar_add(out=den, in0=den, scalar1=1e-30)
                    rden = small.tile([128, GRP], mybir.dt.float32, tag="w2")
                    nc.vector.reciprocal(out=rden, in_=den)
                    q = small.tile([128, GRP], mybir.dt.float32, tag="w3")
                    nc.vector.tensor_mul(out=q, in0=f4, in1=rden)
                    qd = small.tile([128, GRP], mybir.dt.float32, tag="w4")
                    nc.vector.tensor_mul(out=qd, in0=q, in1=dtau)
                    ts_ = small.tile([128, GRP], mybir.dt.float32, tag="w5")
                    nc.vector.tensor_add(out=ts_, in0=qd, in1=c["tm"])
                    tc_ = small.tile([128, GRP], mybir.dt.float32, tag="w6")
                    nc.vector.tensor_max(out=tc_, in0=ts_, in1=c["tlo"])
                    tau_m2 = small.tile([128, GRP], mybir.dt.float32, tag="tm")
                    nc.vector.tensor_tensor(
                        out=tau_m2, in0=tc_, in1=c["hi"], op=_OP.min
                    )
                c["tm_prev"], c["f_prev"] = c["tm"], f4
                c["tm"] = tau_m2

            def emit_tail(gi):
                """Normalize + store the final eval's p (after the next pair's
                first relus, so those hit the DVE queue first)."""
                s, t0 = groups[gi]
                c = st[gi]
                p_ts, s4 = c["p_ts"], c["s4"]
                r4 = small.tile([128, GRP], mybir.dt.float32, tag="r")
                nc.vector.reciprocal(out=r4, in_=s4)
                for j in range(GRP):
                    rows = slice((t0 + j) * 128, (t0 + j + 1) * 128)
                    nc.vector.tensor_scalar_mul(
                        out=p_ts[j], in0=p_ts[j], scalar1=r4[:, j : j + 1]
                    )
                    nc.sync.dma_start(out=out[s, rows, :], in_=p_ts[j])
                del st[gi]

            # Pairwise interleave: groups (2k, 2k+1) advance eval-by-eval so
            # one group's DVE phase hides under the other's ACT phase.
            # Staggered startup: group 0's DMA+prep+relu go first so the first
            # Ln is not gated on the whole initial DMA burst.
            emit_dma(0)
            emit_prep(0)
            emit_relu(0, 1)
            emit_dma(1)
            emit_prep(1)
            emit_relu(1, 1)
            emit_dma(2)
            emit_dma(3)
            n_g = len(groups)
            for k in range(0, n_g, 2):
                for g in (k + 4, k + 5):  # two pairs ahead of their prep
                    if g < n_g:
                        emit_dma(g)
                for i in range(1, n_evals + 1):
                    if i > 1:
                        emit_relu(k, i)
                    emit_rest(k, i)
                    if i > 1:
                        emit_relu(k + 1, i)
                    emit_rest(k + 1, i)
                    if i == 2:
                        for g in (k + 2, k + 3):
                            if g < n_g:
                                emit_prep(g)
                # next pair's first relus outrank the normalize tails on DVE
                if k + 2 < n_g:
                    emit_relu(k + 2, 1)
                    emit_relu(k + 3, 1)
                emit_tail(k)
                emit_tail(k + 1)

    nc.finalize()
    return nc


_NC_CACHE = {}


def _get_nc():
    key = (N_BIS, N_SEC)
    if key not in _NC_CACHE:
        _NC_CACHE[key] = _build_nc(*key)
    return _NC_CACHE[key]


def _host_scal_table(alpha: np.ndarray) -> np.ndarray:
    """Per-(b,h)-slice constant table (x-space), f32 math."""
    a = (F32(1.01) + F32(0.98) * alpha.astype(F32)).astype(F32)  # [H]
    am1 = (a - F32(1.0)).astype(F32)
    inv = (F32(1.0) / am1).astype(F32)
    dm0p = ((F32(1.0) - (F32(1.0 / K) ** inv)) * inv).astype(F32)  # x-space width
    tab = np.zeros((B * H, M_COLS), dtype=F32)
    for g in range(B * H):
        h = g % H
        dms = [F32(dm0p[h] * F32(0.5) ** i) for i in range(1, N_EVALS + 2)]
        tab[g, 0] = am1[h]
        tab[g, 1] = inv[h]
        tab[g, 2] = -inv[h]
        tab[g, 3] = F32(dms[0] - inv[h])  # c0 = dm_1 - inv
        for i in range(1, N_EVALS + 1):
            tab[g, 3 + i] = dms[i - 1]
    return tab


def kernel(att_scores: np.ndarray, alpha: np.ndarray, **run_kwargs) -> np.ndarray:
    assert att_scores.shape == (B, H, Q, K), att_scores.shape
    nc = _get_nc()
    xr = np.ascontiguousarray(att_scores, dtype=np.float32).reshape(B * H, Q, K)
    tab = _host_scal_table(np.asarray(alpha))
    in_maps = []
    for c in range(N_CORES):
        sl = slice(c * SLICES_PER_CORE, (c + 1) * SLICES_PER_CORE)
        in_maps.append(
            {
                "x": np.ascontiguousarray(xr[sl]),
                "scal": np.ascontiguousarray(tab[sl]),
            }
        )
    res = run_bass_kernel_spmd(nc, in_maps, core_ids=list(range(N_CORES)), **run_kwargs)
    outs = np.stack([res.results[c]["out"] for c in range(N_CORES)], axis=0)
    full = outs.reshape(B, H, Q, K).astype(np.float32)
    if run_kwargs:
        kernel.last_result = res
    return full
